# revision 43
# baseline (speedup 1.0000x reference)
"""LoRA-with-routing kernel for Trainium2 (8 NeuronCores, SPMD).

out[b] = base[b] + (x[b] @ lora_A[idx[b]]) @ lora_B[idx[b]] * s[idx[b]]

Sharding: data-parallel over batch (B=8 rows, one per core). The adapter
gather (routing) happens host-side while sharding: each core receives its
batch row plus that row's adapter weights (scale folded into B).

HBM traffic is the bottleneck (ridge regime), so IO is minimized:
x and lora_A are fp8e4m3 (GEMM1 runs in DoubleRow perf mode, contracting
256 rows per pass), base is bf16, out is stored bf16 and upcast to f32
host-side. All tensors are pre-tiled host-side so every DMA line is
2-8KB contiguous per partition.

Device pipeline per core (T=2048, D=4096, R=64), per 512-token group:
  1. load 2 x slabs [128p, 16kt, 512t] fp8 (1 MiB each)
  2. GEMM1 (PE, fp8 DoubleRow): interT[64r, 512t] += A.T @ xT, 16 passes
  3. ACT evac interT -> bf16 SBUF
  4. per 128-token subtile: load base bf16, GEMM2 y[128,512] = interT.T @ B
     (bf16), add into base alternating DVE/Pool, store bf16
"""

import sys

for _p in ("/opt/trn_rl_repo", "/root/.axon_site/_ro/trn_rl_repo"):
    if _p not in sys.path:
        sys.path.append(_p)

import numpy as np
import ml_dtypes

import concourse.bass as bass
import concourse.bacc as bacc
import concourse.mybir as mybir
from concourse import tile

B, T, D, R = 8, 2048, 4096, 64
P = 128          # partitions
KT = D // P      # 32 k-tiles (contraction, 128 each)
TG = 512         # token group (GEMM1 moving dim, one PSUM bank of f32)
OCH = 512        # output free chunk (one PSUM bank of f32)
OC = D // OCH    # 8 o-chunks
KSLAB = 16       # k-tiles per x slab (1 MiB fp8 transfers, 8KB/partition)

F32 = mybir.dt.float32
BF16 = mybir.dt.bfloat16
FP8 = mybir.dt.float8e4
NP_FP8 = ml_dtypes.float8_e4m3fn


def build_program(t_tokens: int = T):
    ng = t_tokens // TG
    nslab = ng * (KT // KSLAB)
    nc = bacc.Bacc("TRN2", target_bir_lowering=False, debug=False, num_devices=B)
    xt = nc.dram_tensor("xt", [nslab, P, KSLAB, TG], FP8, kind="ExternalInput").ap()
    base = nc.dram_tensor("base", [t_tokens, D], BF16, kind="ExternalInput").ap()
    a_w = nc.dram_tensor("a_w", [P, KT, R], FP8, kind="ExternalInput").ap()
    b_w = nc.dram_tensor("b_w", [R, D], BF16, kind="ExternalInput").ap()
    out = nc.dram_tensor("out", [t_tokens, D], BF16, kind="ExternalOutput").ap()

    with tile.TileContext(nc) as tc:
        _body(tc, xt, base, a_w, b_w, out, ng)
    nc.compile()
    return nc


def _body(tc, xt, base, a_w, b_w, out, ng):
    nc = tc.nc
    nh = KT // KSLAB  # x slabs per token group
    nsub = TG // P
    with (
        tc.tile_pool(name="const", bufs=1) as cpool,
        tc.tile_pool(name="xc", bufs=4) as xc_pool,
        tc.tile_pool(name="bs", bufs=10) as bs_pool,
        tc.tile_pool(name="it", bufs=2) as it_pool,
        tc.tile_pool(name="ysb", bufs=8) as y_pool,
        tc.tile_pool(name="ps1", bufs=2, space="PSUM") as ps1,
        tc.tile_pool(name="ps2", bufs=6, space="PSUM") as ps2,
    ):
        # Adapter weights, loaded once (host pre-tiled) — on the scalar
        # queue so the first x/base loads on sync aren't serialized behind.
        # a_sb[p, kt, r] = A[kt*128 + p, r]  (contraction dim on partitions)
        a_sb = cpool.tile([P, KT, R], FP8)
        nc.scalar.dma_start(a_sb[:], a_w[:])
        # b_sb[r, o] on partitions 0..63
        b_sb = cpool.tile([R, D], BF16)
        nc.scalar.dma_start(b_sb[:], b_w[:])

        for g in range(ng):
            t0 = g * TG
            # Prefetch this group's x slabs and base subtiles up front; all
            # loads ride the sync queue, which never waits on compute (only
            # on pool-buffer reuse), so DMA streams ahead of the engines.
            xcs = []
            for h in range(nh):
                xc = xc_pool.tile([P, KSLAB, TG], FP8)
                nc.sync.dma_start(xc[:], xt[g * nh + h])
                xcs.append(xc)
            bss = []
            for sub in range(nsub):
                bs = bs_pool.tile([P, D], BF16)
                # group 0's base rides the (empty-at-t0) scalar queue too, so
                # two queues issue load triggers during the startup ramp
                ld_eng = nc.scalar if g == 0 else nc.sync
                ld_eng.dma_start(bs[:], base[t0 + sub * P : t0 + (sub + 1) * P, :])
                bss.append(bs)

            # GEMM1: interT[r, t] = sum_kt A_kt.T @ xT_kt, accumulated in
            # PSUM, fp8 DoubleRow (2 k-tiles = 256 contraction rows/pass).
            it_ps = ps1.tile([R, TG], F32)
            for h in range(nh):
                for i in range(KSLAB // 2):
                    kt = h * KSLAB + 2 * i
                    nc.tensor.matmul(
                        it_ps[:],
                        a_sb[:, kt : kt + 2, :],
                        xcs[h][:, 2 * i : 2 * i + 2, :],
                        start=(kt == 0),
                        stop=(kt == KT - 2),
                        perf_mode=mybir.MatmulPerfMode.DoubleRow,
                    )

            # evacuate to bf16 (GEMM2 stationary operand) on ACT
            it_sb = it_pool.tile([R, TG], BF16)
            nc.scalar.copy(it_sb[:], it_ps[:])

            for sub in range(nsub):
                tt = t0 + sub * P
                bs = bss[sub]
                last_tile = g == ng - 1 and sub == nsub - 1
                for o in range(OC):
                    y_ps = ps2.tile([P, OCH], F32)
                    nc.tensor.matmul(
                        y_ps[:],
                        it_sb[:, sub * P : (sub + 1) * P],
                        b_sb[:, o * OCH : (o + 1) * OCH],
                        start=True,
                        stop=True,
                    )
                    dst = bs[:, o * OCH : (o + 1) * OCH]
                    # Measured: DVE PSUM-add 684ns, Pool SBUF-add 1155ns (+
                    # ACT 687ns feeding it, since Pool can't read PSUM and
                    # DVE is the only tensor_tensor engine with PSUM access).
                    # Balance point: ~29% of chunks via the ACT+Pool route.
                    c = sub * OC + o
                    pool_route = c % 7 in (2, 5)
                    if pool_route:
                        y_sb = y_pool.tile([P, OCH], BF16)
                        nc.scalar.copy(y_sb[:], y_ps[:])
                        nc.gpsimd.tensor_add(dst, dst, y_sb[:])
                    else:
                        nc.vector.tensor_add(dst, dst, y_ps[:])
                    if last_tile:
                        # drain the kernel tail: store each o-chunk as soon
                        # as its add lands, alternating queues — the ~650ns
                        # per-trigger issue cost is the tail's critical path
                        st = nc.scalar if o % 2 == 0 else nc.gpsimd
                        st.dma_start(
                            out[tt : tt + P, o * OCH : (o + 1) * OCH], dst
                        )
                # Issue the store for the PREVIOUS subtile here: by now its
                # adds have long finished, so the trigger doesn't stall the
                # Pool queue (a same-sub store waits on DVE's adds and blocks
                # the Pool adds queued behind it).
                prev = g * nsub + sub - 1
                if prev >= 0:
                    ptt = (prev // nsub) * TG + (prev % nsub) * P
                    pbs = bss if prev // nsub == g else bss_prev
                    # alternate store queues (SWDGE/HWDGE): parallel
                    # descriptor generation and a cheaper end-of-kernel drain
                    st_eng = nc.gpsimd if prev % 2 == 0 else nc.scalar
                    st_eng.dma_start(out[ptt : ptt + P, :], pbs[prev % nsub][:])
            bss_prev = bss


def _tile_x(xrow: np.ndarray) -> np.ndarray:
    """[T, D] f32 -> [nslab, P, KSLAB, TG] fp8 with d = kt*128 + p.

    Slab s = g*(KT//KSLAB) + h holds tokens [g*TG, (g+1)*TG) and k-tiles
    [h*KSLAB, (h+1)*KSLAB); each partition line is KSLAB*TG contiguous.
    """
    ng = xrow.shape[0] // TG
    nh = KT // KSLAB
    xr = xrow.reshape(ng, TG, KT, P)              # [g, t, kt, p]
    xr = xr.reshape(ng, TG, nh, KSLAB, P)
    xr = xr.transpose(0, 2, 4, 3, 1)              # [g, h, p, ktl, t]
    return np.ascontiguousarray(
        xr.reshape(ng * nh, P, KSLAB, TG)
    ).astype(NP_FP8)


def shard_inputs(x, base_output, adapter_indices, lora_A, lora_B, lora_scaling):
    idx = np.asarray(adapter_indices).astype(np.int64)
    a_b = np.asarray(lora_A, dtype=np.float32)[idx]        # [B, D, R]
    b_b = np.asarray(lora_B, dtype=np.float32)[idx]        # [B, R, D]
    s_b = np.asarray(lora_scaling, dtype=np.float32)[idx]  # [B]
    # A ~ U(-1/64, 1/64) sits entirely in e4m3's denormal range; scale it
    # up by 32 (into normals) and fold 1/32 into the bf16 B for free.
    b_scaled = (b_b * (s_b[:, None, None] / 32.0)).astype(ml_dtypes.bfloat16)
    # a_w[p, kt, r] = 32 * A[kt*128 + p, r]
    a_t = np.ascontiguousarray(
        (a_b * 32.0).reshape(B, KT, P, R).transpose(0, 2, 1, 3)
    ).astype(NP_FP8)
    xs = np.asarray(x, dtype=np.float32)
    bs = np.asarray(base_output, dtype=np.float32)
    return [
        {
            "xt": _tile_x(xs[b]),
            "base": np.ascontiguousarray(bs[b]).astype(ml_dtypes.bfloat16),
            "a_w": a_t[b],
            "b_w": np.ascontiguousarray(b_scaled[b]),
        }
        for b in range(B)
    ]


def run(inputs: dict, trace: bool = False, **kwargs):
    """Build + run on 8 cores. Returns (output [B,T,D] f32, BassKernelResults)."""
    from concourse.bass_utils import run_bass_kernel_spmd

    nc = build_program()
    in_maps = shard_inputs(**inputs)
    res = run_bass_kernel_spmd(
        nc, in_maps, core_ids=list(range(B)), trace=trace, **kwargs
    )
    out = np.stack(
        [res.results[b]["out"].astype(np.float32) for b in range(B)], axis=0
    )
    return out, res


def kernel(x, base_output, adapter_indices, lora_A, lora_B, lora_scaling):
    out, _ = run(
        dict(
            x=x,
            base_output=base_output,
            adapter_indices=adapter_indices,
            lora_A=lora_A,
            lora_B=lora_B,
            lora_scaling=lora_scaling,
        )
    )
    return out
